# revision 2
# baseline (speedup 1.0000x reference)
"""Trainium2 Bass kernel for EnhancedTransformerNetwork (local rel-pos attention + global MHA).

Sharding: data-parallel over batch. 8 cores x 4 batches each; weights replicated.

Per-core algorithm (tokens = 4*512 = 2048, all matmuls bf16, PSUM f32):
  - inputs cast to bf16, transposed to [din, tokens] via XBAR DMA transpose
  - local:  qT8 = (wq/8)^T x^T (T-layout), kT (T), v natural.
            rel bias: T2e[lc] = q8_chunk @ rel_ext window (clip baked into the
            host-built rel_ext table), round-trip bf16 through DRAM, read back
            with a diagonal access pattern (stride W-1 skews rows) -> bias[l,r].
            scores psum = ident-matmul(bias) + qT8^T kT; exp on ACT with
            accum_out row sums; normalize exp tiles by 0.7/sums (per-partition,
            on Pool); XBAR-transpose -> expT; AV with v as lhsT, expT as rhs
            -> localT [hd, l] (T-layout).
  - global: scoresT[r,l] = gkT^T gqT (K=128), exp; sums via ones-matmul;
            recip broadcast via fp32 K=1 PE outer product; AV with vg as lhsT
            -> gT [hdg, l] T-layout; g_out proj in T-layout (0.3 folded).
  - combT = localT + gprojT, final wo matmul, DMA out.
"""

import numpy as np
import ml_dtypes

import concourse.bass as bass
import concourse.mybir as mybir
import concourse.tile as tile
from concourse.bass_utils import run_bass_kernel_spmd

BF = mybir.dt.bfloat16
F32 = mybir.dt.float32
AF = mybir.ActivationFunctionType
ALU = mybir.AluOpType

D = 512
H = 8
HD = 64
HG = 4
HDG = 128
MAXREL = 256
S = 512
B = 32
NCORES = 8
BL = B // NCORES          # batches per core
TOK = BL * S              # tokens per core
EXTW = 1025               # rel_ext width: u' in [0, 1024], clip baked in
T2W = 640                 # per-chunk T2e width (639 used, padded to 640)

_cache = {}


def _split_wide_waits(nc, max_waits=1):
    """This walrus build supports only one sem wait per instruction; Tile's
    final drain carries one wait per logical proc. Hoist excess waits onto
    preceding same-engine NOPs (same-engine program order keeps semantics)."""
    f = nc.m.functions[0]
    created = {}
    for bb in f.blocks:
        for inst in bb.instructions:
            si = inst.sync_info
            if si is not None and len(si.on_wait) > max_waits:
                waits = list(si.on_wait)
                head, tail = waits[:-max_waits], waits[-max_waits:]
                nops = []
                for i in range(0, len(head), max_waits):
                    nop = nc.engines[inst.engine].nop(nofuse=True)
                    nop.ins.sync_info = mybir.SyncInfo(
                        on_wait=head[i:i + max_waits], on_update=[])
                    nops.append(nop.ins)
                si.on_wait = tail
                created[inst.name] = nops
    if not created:
        return
    names = {n.name for nops in created.values() for n in nops}
    for bb in f.blocks:
        out = []
        for inst in bb.instructions:
            if inst.name in names:
                continue
            if inst.name in created:
                out.extend(created[inst.name])
            out.append(inst)
        bb.instructions = out


def _build(flags, reps=1):
    """flags: (has_bq, has_bk, has_bv, has_bgq, has_bgk, has_bgv, has_bgo, has_bo)
    reps>1 replicates the whole computation on device (benchmarking only)."""
    has_bq, has_bk, has_bv, has_bgq, has_bgk, has_bgv, has_bgo, has_bo = flags
    nc = bass.Bass("TRN2", target_bir_lowering=False, debug=False,
                   num_devices=NCORES)

    # --- I/O ---
    x_q = nc.dram_tensor("query", [128, 4, TOK], BF, kind="ExternalInput")
    x_k = nc.dram_tensor("key", [128, 4, TOK], BF, kind="ExternalInput")
    x_v = nc.dram_tensor("value", [128, 4, TOK], BF, kind="ExternalInput")
    cw = {}
    for name in ["cqT", "ckT", "cvT", "cgqT", "cgkT", "cgvT", "cgoT", "cwoT"]:
        cw[name] = nc.dram_tensor(name, [128, 4, D], BF, kind="ExternalInput")
    relx_d = nc.dram_tensor("relx", [128, 1032], BF, kind="ExternalInput")
    ident_d = nc.dram_tensor("ident", [128, 128], BF, kind="ExternalInput")
    pbias_d = {}
    for name, has in [("bqT", has_bq), ("bkT", has_bk), ("bvT", has_bv),
                      ("bgqT", has_bgq), ("bgkT", has_bgk)]:
        if has:
            pbias_d[name] = nc.dram_tensor(name, [128, 4], F32, kind="ExternalInput")
    fbias_d = {}
    for name, has in [("bgvb", has_bgv), ("bgobT", False), ("bob", has_bo)]:
        if has:
            fbias_d[name] = nc.dram_tensor(name, [D], F32, kind="ExternalInput")
    if has_bgo:
        pbias_d["bgoT"] = nc.dram_tensor("bgoT", [128, 4], F32, kind="ExternalInput")
    out_d = nc.dram_tensor("out", [TOK, D], F32, kind="ExternalOutput")

    # DRAM scratch for the T2e diagonal round trip, one per (b, h, lc)
    t2scr = [[[nc.dram_tensor(f"t2_{b}_{h}_{lc}", [128, T2W], BF)
               for lc in range(4)] for h in range(H)] for b in range(BL)]

    with tile.TileContext(nc) as tc:
        import contextlib
        ctx = contextlib.ExitStack()
        with ctx:
            singles = ctx.enter_context(tc.tile_pool(name="singles", bufs=1))
            pers = ctx.enter_context(tc.tile_pool(name="pers", bufs=2))
            perh = ctx.enter_context(tc.tile_pool(name="perh", bufs=2))
            xload = ctx.enter_context(tc.tile_pool(name="xload", bufs=4))
            trans = ctx.enter_context(tc.tile_pool(name="trans", bufs=4))
            ps512 = ctx.enter_context(tc.tile_pool(name="ps512", bufs=2, space="PSUM"))
            pswide = ctx.enter_context(tc.tile_pool(name="pswide", bufs=1, space="PSUM"))
            psav = ctx.enter_context(tc.tile_pool(name="psav", bufs=1, space="PSUM"))
            pstr = ctx.enter_context(tc.tile_pool(name="pstr", bufs=2, space="PSUM"))
            pssm = ctx.enter_context(tc.tile_pool(name="pssm", bufs=1, space="PSUM"))

            # --- constants ---
            w_sb = {}
            for name in cw:
                t = singles.tile([128, 4, D], BF, tag=name)
                nc.sync.dma_start(t[:], cw[name][:, :, :])
                w_sb[name] = t
            relx = singles.tile([128, 1032], BF, tag="relx")
            nc.sync.dma_start(relx[:], relx_d[:, :])
            ident = singles.tile([128, 128], BF, tag="ident")
            nc.sync.dma_start(ident[:], ident_d[:, :])
            ones_bf = singles.tile([128, 1], BF, tag="ones_bf")
            nc.vector.memset(ones_bf[:], 1.0)
            ones_f32 = singles.tile([1, 128], F32, tag="ones_f32")
            nc.vector.memset(ones_f32[:], 1.0)
            ones_bfr = singles.tile([1, 128], BF, tag="ones_bfr")
            nc.vector.memset(ones_bfr[:], 1.0)
            pb_sb = {}
            for name, t in pbias_d.items():
                s = singles.tile([128, 4], F32, tag=name)
                nc.sync.dma_start(s[:], t[:, :])
                pb_sb[name] = s
            fb_sb = {}
            for name, t in fbias_d.items():
                s = singles.tile([128, D], F32, tag=name)
                nc.sync.dma_start(s[:], bass.AP(t, 0, [[0, 128], [1, D]]))
                fb_sb[name] = s

            evac_ctr = [0]

            def evac_copy(dst, src, bias_col=None):
                """PSUM->SBUF copy (+ optional per-partition bias), alternating
                ACT/DVE."""
                if bias_col is not None:
                    nc.vector.tensor_scalar_add(dst, src, bias_col)
                    return
                if evac_ctr[0] % 3 == 0:
                    nc.scalar.activation(dst, src, AF.Copy)
                else:
                    nc.vector.tensor_copy(dst, src)
                evac_ctr[0] += 1

            def xbar_T(dst_3d, src_2d, n=4):
                """Transpose n [128,128] bf16 blocks of src [128, n*128] into
                dst [128, n, 128] via PE transpose + one grouped strided
                copyback (ACT/DVE alternating)."""
                ptr = pstr.tile([128, 4, 128], BF, tag="tr4")
                for j in range(n):
                    nc.tensor.transpose(ptr[:, j, :], src_2d[:, j * 128:(j + 1) * 128], ident[:])
                evac_copy(dst_3d[:, :, :], ptr[:, :n, :])

            for rep in range(reps):
              for b in range(BL):
                tok0 = b * S

                # ---- inputs arrive host-transposed: xT [128(din%128), 4(ic), tok]
                xT = {}
                for key, xd in (("q", x_q), ("k", x_k), ("v", x_v)):
                    xT_t = pers.tile([128, 4, S], BF, tag=f"xT_{key}")
                    nc.sync.dma_start(xT_t[:], xd[:, :, tok0:tok0 + S])
                    xT[key] = xT_t

                # ---- projections
                def projT(dst, wname, xkey, bias_name):
                    # dst [128(dout%128), 4(oc), 512(l)]  (T-layout out)
                    for oc in range(4):
                        ps = ps512.tile([128, S], F32, tag="mm512")
                        for ic in range(4):
                            nc.tensor.matmul(ps[:], w_sb[wname][:, ic, oc * 128:(oc + 1) * 128],
                                             xT[xkey][:, ic, :], start=(ic == 0), stop=(ic == 3))
                        bias_col = pb_sb[bias_name][:, oc:oc + 1] if bias_name in pb_sb else None
                        evac_copy(dst[:, oc, :], ps[:], bias_col)

                def projN(dst, wname, xkey, bias_name):
                    # dst [128(tok%128), 4(tc), 512(dout)]  (natural out)
                    # (bias along free dim is deferred to the attention output)
                    for tck in range(4):
                        ps = ps512.tile([128, D], F32, tag="mm512")
                        for ic in range(4):
                            nc.tensor.matmul(ps[:], xT[xkey][:, ic, tck * 128:(tck + 1) * 128],
                                             w_sb[wname][:, ic, :], start=(ic == 0), stop=(ic == 3))
                        evac_copy(dst[:, tck, :], ps[:])

                qT8 = pers.tile([128, 4, S], BF, tag="qT8")
                kT = pers.tile([128, 4, S], BF, tag="kT")
                gqT = pers.tile([128, 4, S], BF, tag="gqT")
                gkT = pers.tile([128, 4, S], BF, tag="gkT")
                v_n = pers.tile([128, 4, D], BF, tag="v_n")
                vg_n = pers.tile([128, 4, D], BF, tag="vg_n")
                projT(qT8, "cqT", "q", "bqT")
                projT(kT, "ckT", "k", "bkT")
                projT(gqT, "cgqT", "q", "bgqT")
                projT(gkT, "cgkT", "k", "bgkT")
                projN(v_n, "cvT", "v", None)
                projN(vg_n, "cgvT", "v", None)

                localT = pers.tile([128, 4, S], BF, tag="localT")

                # ---- local attention ----
                for h in range(H):
                    oc, po = h // 2, (h % 2) * 64
                    expT = perh.tile([128, 4, S], BF, tag="expT")
                    for lc in range(4):
                        qs = qT8[po:po + 64, oc, lc * 128:(lc + 1) * 128]  # lhsT [64,128]
                        # T2e chunk
                        u0 = 385 - lc * 128
                        pw = pswide.tile([128, 1024], F32, tag="t2psum")
                        nc.tensor.matmul(pw[:, 0:512], qs, relx[po:po + 64, u0:u0 + 512], start=True, stop=True)
                        nc.tensor.matmul(pw[:, 512:640], qs, relx[po:po + 64, u0 + 512:u0 + 640], start=True, stop=True)
                        t2sb = trans.tile([128, T2W], BF, tag="t2sb")
                        evac_copy(t2sb[:], pw[:, 0:T2W])
                        nc.sync.dma_start(t2scr[b][h][lc][:, :], t2sb[:])
                        bias_sb = trans.tile([128, S], BF, tag="bias_sb")
                        nc.sync.dma_start(bias_sb[:], bass.AP(t2scr[b][h][lc], 127, [[T2W - 1, 128], [1, S]]))
                        # scores = q k^T / 8 + bias; alternate the bias add
                        # between a PE identity-matmul accumulate and a DVE add
                        ps = ps512.tile([128, S], F32, tag="mm512")
                        if (h * 4 + lc) % 2 == 0:
                            nc.tensor.matmul(ps[:], ident[:], bias_sb[:], start=True, stop=False)
                            nc.tensor.matmul(ps[:], qs, kT[po:po + 64, oc, :], start=False, stop=True)
                        else:
                            nc.tensor.matmul(ps[:], qs, kT[po:po + 64, oc, :], start=True, stop=True)
                            nc.vector.tensor_tensor(ps[:], ps[:], bias_sb[:], ALU.add)
                        exp_sb = trans.tile([128, S], BF, tag="exp_sb")
                        sums = trans.tile([128, 1], F32, tag="sums")
                        nc.scalar.activation(exp_sb[:], ps[:], AF.Exp, accum_out=sums[:])
                        recip = trans.tile([128, 1], F32, tag="recip")
                        nc.vector.reciprocal(recip[:], sums[:])
                        # normalize by 0.7/sums on Pool (idle engine), bf16
                        expn = trans.tile([128, S], BF, tag="expn")
                        nc.gpsimd.tensor_scalar(expn[:], exp_sb[:], recip[:, 0:1], 0.7,
                                                ALU.mult, ALU.mult)
                        xbar_T(expT[:, :, lc * 128:(lc + 1) * 128], expn[:])
                    # AV: psum [hd, l] accumulated over r-chunks; expT is rhs
                    pav = psav.tile([64, S], F32, tag="avT")
                    for rc in range(4):
                        nc.tensor.matmul(pav[:], v_n[:, rc, h * 64:(h + 1) * 64],
                                         expT[:, rc, :], start=(rc == 0), stop=(rc == 3))
                    bias_col = pb_sb["bvT"][po:po + 64, oc:oc + 1] if "bvT" in pb_sb else None
                    if bias_col is not None:
                        # deferred v bias (scaled 0.7 on host): out = AV*0.7/sums + 0.7*bv
                        nc.vector.tensor_scalar_add(localT[po:po + 64, oc, :], pav[:], bias_col)
                    else:
                        evac_copy(localT[po:po + 64, oc, :], pav[:])

                # ---- global attention ----
                gT = pers.tile([128, 4, S], BF, tag="gT")
                for hg in range(HG):
                    expTg = perh.tile([128, 4, S], BF, tag="expTg")
                    for rc in range(4):
                        ps = ps512.tile([128, S], F32, tag="mm512")
                        nc.tensor.matmul(ps[:], gkT[:, hg, rc * 128:(rc + 1) * 128],
                                         gqT[:, hg, :], start=True, stop=True)
                        nc.scalar.activation(expTg[:, rc, :], ps[:], AF.Exp)
                    psum = pssm.tile([1, S], F32, tag="small")
                    for rc in range(4):
                        nc.tensor.matmul(psum[:], ones_bf[:], expTg[:, rc, :],
                                         start=(rc == 0), stop=(rc == 3))
                    recip_g = trans.tile([1, S], BF, tag="recip_g")
                    with nc.allow_low_precision(reason="bf16 recip feeds bf16 bcast matmul; error cancels with bf16 exp sums"):
                        nc.vector.reciprocal(recip_g[:], psum[:])
                    pbc = ps512.tile([128, S], F32, tag="mm512")
                    nc.tensor.matmul(pbc[:], ones_bfr[:], recip_g[:], start=True, stop=True)
                    recipB = trans.tile([128, S], F32, tag="recipB")
                    nc.scalar.activation(recipB[:], pbc[:], AF.Copy)
                    pgT = ps512.tile([128, S], F32, tag="mm512")
                    for rc in range(4):
                        nc.tensor.matmul(pgT[:], vg_n[:, rc, hg * 128:(hg + 1) * 128],
                                         expTg[:, rc, :], start=(rc == 0), stop=(rc == 3))
                    nc.vector.tensor_tensor(gT[:, hg, :], pgT[:], recipB[:], ALU.mult)
                    if "bgvb" in fb_sb:
                        nc.vector.tensor_scalar_add(gT[:, hg, :], gT[:, hg, :],
                                                    fb_sb["bgvb"][hg * 128:(hg + 1) * 128, 0:1])

                # ---- g_out proj (T-layout, x0.3 folded) + combine with localT ----
                combT = pers.tile([128, 4, S], BF, tag="combT")
                for oc in range(4):
                    ps = ps512.tile([128, S], F32, tag="mm512")
                    for ic in range(4):
                        nc.tensor.matmul(ps[:], w_sb["cgoT"][:, ic, oc * 128:(oc + 1) * 128],
                                         gT[:, ic, :], start=(ic == 0), stop=(ic == 3))
                    if "bgoT" in pb_sb:
                        nc.vector.tensor_scalar_add(ps[:], ps[:], pb_sb["bgoT"][:, oc:oc + 1])
                    nc.vector.tensor_tensor(combT[:, oc, :], ps[:], localT[:, oc, :], ALU.add)

                # ---- final wo ----
                for tck in range(4):
                    ps = ps512.tile([128, D], F32, tag="mm512")
                    for ic in range(4):
                        nc.tensor.matmul(ps[:], combT[:, ic, tck * 128:(tck + 1) * 128],
                                         w_sb["cwoT"][:, ic, :], start=(ic == 0), stop=(ic == 3))
                    osb = trans.tile([128, D], F32, tag="osb")
                    if "bob" in fb_sb:
                        nc.vector.tensor_tensor(osb[:], ps[:], fb_sb["bob"][:], ALU.add)
                    else:
                        evac_copy(osb[:], ps[:])
                    nc.sync.dma_start(out_d[tok0 + tck * 128: tok0 + (tck + 1) * 128, :], osb[:])

    _split_wide_waits(nc)
    return nc


def _arr_T(w, scale=1.0):
    """torch Linear weight [dout, din] -> lhsT const [128, 4, dout] bf16 with
    [p, ic, o] = w[o, ic*128+p]*scale."""
    wt = (w.T.astype(np.float64) * scale).astype(np.float32)
    return np.ascontiguousarray(
        wt.reshape(4, 128, wt.shape[1]).transpose(1, 0, 2)).astype(ml_dtypes.bfloat16)


def _arr_bias(bvec, scale=1.0):
    """bias [512] -> [128, 4] f32 with [p, oc] = b[oc*128+p]*scale."""
    return np.ascontiguousarray(
        (np.asarray(bvec) * scale).astype(np.float32).reshape(4, 128).T)


def kernel(query, key, value, wq, bq, wk, bk, wv, bv, wo, bo, rel_k,
           g_in_w, g_in_b, g_out_w, g_out_b):
    query = np.asarray(query, dtype=np.float32)
    key = np.asarray(key, dtype=np.float32)
    value = np.asarray(value, dtype=np.float32)

    sg = 1.0 / np.sqrt(np.float32(HDG))
    consts = {
        "cqT": _arr_T(np.asarray(wq), 0.125),
        "ckT": _arr_T(np.asarray(wk)),
        "cvT": _arr_T(np.asarray(wv)),
        "cgqT": _arr_T(np.asarray(g_in_w)[:D], sg),
        "cgkT": _arr_T(np.asarray(g_in_w)[D:2 * D]),
        "cgvT": _arr_T(np.asarray(g_in_w)[2 * D:]),
        "cgoT": _arr_T(np.asarray(g_out_w), 0.3),
        "cwoT": _arr_T(np.asarray(wo)),
        "ident": np.eye(128, dtype=ml_dtypes.bfloat16),
    }
    # rel_ext [128, 1032]: rows 0-63 and 64-127 both hold
    # [d, u] = 8*rel_k[clip(u-256, 0, 512), d] (duplicated so lhsT/rhs base
    # partitions match for odd heads)
    idx = np.clip(np.arange(EXTW) - 256, 0, 2 * MAXREL)
    tbl = (8.0 * np.asarray(rel_k, dtype=np.float32)[idx]).T  # [64, 1025]
    relx = np.zeros((128, 1032), dtype=ml_dtypes.bfloat16)
    relx[:HD, :EXTW] = tbl.astype(ml_dtypes.bfloat16)
    relx[HD:2 * HD, :EXTW] = tbl.astype(ml_dtypes.bfloat16)
    consts["relx"] = relx

    bq, bk, bv = np.asarray(bq), np.asarray(bk), np.asarray(bv)
    bo, g_in_b, g_out_b = np.asarray(bo), np.asarray(g_in_b), np.asarray(g_out_b)
    flags = (bool(np.any(bq)), bool(np.any(bk)), bool(np.any(bv)),
             bool(np.any(g_in_b[:D])), bool(np.any(g_in_b[D:2 * D])),
             bool(np.any(g_in_b[2 * D:])), bool(np.any(g_out_b)), bool(np.any(bo)))
    if flags[0]:
        consts["bqT"] = _arr_bias(bq, 0.125)
    if flags[1]:
        consts["bkT"] = _arr_bias(bk)
    if flags[2]:
        consts["bvT"] = _arr_bias(bv, 0.7)   # deferred past softmax, 0.7 fold
    if flags[3]:
        consts["bgqT"] = _arr_bias(g_in_b[:D], sg)
    if flags[4]:
        consts["bgkT"] = _arr_bias(g_in_b[D:2 * D])
    if flags[5]:
        consts["bgvb"] = g_in_b[2 * D:].astype(np.float32)
    if flags[6]:
        consts["bgoT"] = _arr_bias(g_out_b, 0.3)
    if flags[7]:
        consts["bob"] = bo.astype(np.float32)

    if flags not in _cache:
        _cache[flags] = _build(flags)
    nc = _cache[flags]

    def _xT(x_core):
        # [BL, S, D] f32 -> [128, 4(ic), TOK] bf16 with [p, ic, t] = x[t, ic*128+p]
        xt = x_core.reshape(TOK, D).T.astype(ml_dtypes.bfloat16)   # [D, TOK]
        return np.ascontiguousarray(xt.reshape(4, 128, TOK).transpose(1, 0, 2))

    in_maps = []
    for c in range(NCORES):
        m = dict(consts)
        m["query"] = _xT(query[c * BL:(c + 1) * BL])
        m["key"] = _xT(key[c * BL:(c + 1) * BL])
        m["value"] = _xT(value[c * BL:(c + 1) * BL])
        in_maps.append(m)

    res = run_bass_kernel_spmd(nc, in_maps, core_ids=list(range(NCORES)))
    global LAST_RESULT
    LAST_RESULT = res
    out = np.concatenate(
        [r["out"].reshape(BL, S, D) for r in res.results], axis=0)
    return out.astype(np.float32)


LAST_RESULT = None



# revision 3
# speedup vs baseline: 1.0173x; 1.0173x over previous
"""Trainium2 Bass kernel for EnhancedTransformerNetwork (local rel-pos attention + global MHA).

Sharding: data-parallel over batch. 8 cores x 4 batches each; weights replicated.

Per-core algorithm (tokens = 4*512 = 2048, all matmuls bf16, PSUM f32):
  - inputs arrive host-transposed bf16 [din, tokens]
  - local:  qT8 = (wq/8)^T x^T (T-layout), kT (T), v natural.
            Per (batch, l-chunk): rel bias T2e for all 8 heads, with even/odd
            head matmuls row-tiled on array halves (K=64), evac'd into one
            wide tile, ONE DRAM write + ONE 3D diagonal-AP read (clip baked
            into the host rel_ext table) -> biasw[l, h, r].
            scores psum = qk (row-tiled pair, start) + ident-matmul(bias);
            exp on ACT with accum_out row sums; batched reciprocal per chunk;
            0.7/sums normalization on Pool; ONE DMA XBAR transpose per
            (batch, l-chunk) turns all 8 heads' exp [l, r] into expT [r, l];
            AV col-tiled per head pair (M=64 halves) -> localT [hd, l].
  - global: scoresT[r,l] = gkT^T gqT (K=128), exp; sums via ones-matmul;
            recip broadcast via fp32 K=1 PE outer product; AV with vg as lhsT
            -> gT [hdg, l] T-layout; g_out proj in T-layout (0.3 folded).
  - combT = localT + gprojT, final wo matmul, DMA out.
"""

import numpy as np
import ml_dtypes

import concourse.bass as bass
import concourse.mybir as mybir
import concourse.tile as tile
from concourse.bass_utils import run_bass_kernel_spmd

BF = mybir.dt.bfloat16
F32 = mybir.dt.float32
AF = mybir.ActivationFunctionType
ALU = mybir.AluOpType

D = 512
H = 8
HD = 64
HG = 4
HDG = 128
MAXREL = 256
S = 512
B = 32
NCORES = 8
BL = B // NCORES          # batches per core
TOK = BL * S              # tokens per core
EXTW = 1025               # rel_ext width: u' in [0, 1024], clip baked in
T2W = 640                 # per-chunk T2e width (639 used, padded to 640)
T2WIDE = H * T2W          # 5120: all 8 heads' T2e rows side by side

_cache = {}


def _split_wide_waits(nc, max_waits=1):
    """This walrus build supports only one sem wait per instruction; Tile's
    final drain carries one wait per logical proc. Hoist excess waits onto
    preceding same-engine NOPs (same-engine program order keeps semantics)."""
    f = nc.m.functions[0]
    created = {}
    for bb in f.blocks:
        for inst in bb.instructions:
            si = inst.sync_info
            if si is not None and len(si.on_wait) > max_waits:
                waits = list(si.on_wait)
                head, tail = waits[:-max_waits], waits[-max_waits:]
                nops = []
                for i in range(0, len(head), max_waits):
                    nop = nc.engines[inst.engine].nop(nofuse=True)
                    nop.ins.sync_info = mybir.SyncInfo(
                        on_wait=head[i:i + max_waits], on_update=[])
                    nops.append(nop.ins)
                si.on_wait = tail
                created[inst.name] = nops
    if not created:
        return
    names = {n.name for nops in created.values() for n in nops}
    for bb in f.blocks:
        out = []
        for inst in bb.instructions:
            if inst.name in names:
                continue
            if inst.name in created:
                out.extend(created[inst.name])
            out.append(inst)
        bb.instructions = out


def _build(flags, reps=1):
    """flags: (has_bq, has_bk, has_bv, has_bgq, has_bgk, has_bgv, has_bgo, has_bo)
    reps>1 replicates the whole computation on device (benchmarking only)."""
    has_bq, has_bk, has_bv, has_bgq, has_bgk, has_bgv, has_bgo, has_bo = flags
    nc = bass.Bass("TRN2", target_bir_lowering=False, debug=False,
                   num_devices=NCORES)

    # --- I/O ---
    x_q = nc.dram_tensor("query", [128, 4, TOK], BF, kind="ExternalInput")
    x_k = nc.dram_tensor("key", [128, 4, TOK], BF, kind="ExternalInput")
    x_v = nc.dram_tensor("value", [128, 4, TOK], BF, kind="ExternalInput")
    cw = {}
    for name in ["cqT", "ckT", "cvT", "cgqT", "cgkT", "cgvT", "cgoT", "cwoT"]:
        cw[name] = nc.dram_tensor(name, [128, 4, D], BF, kind="ExternalInput")
    relx_d = nc.dram_tensor("relx", [128, 1032], BF, kind="ExternalInput")
    ident_d = nc.dram_tensor("ident", [128, 128], BF, kind="ExternalInput")
    pbias_d = {}
    for name, has in [("bqT", has_bq), ("bkT", has_bk), ("bvT", has_bv),
                      ("bgqT", has_bgq), ("bgkT", has_bgk)]:
        if has:
            pbias_d[name] = nc.dram_tensor(name, [128, 4], F32, kind="ExternalInput")
    fbias_d = {}
    for name, has in [("bgvb", has_bgv), ("bob", has_bo)]:
        if has:
            fbias_d[name] = nc.dram_tensor(name, [D], F32, kind="ExternalInput")
    if has_bgo:
        pbias_d["bgoT"] = nc.dram_tensor("bgoT", [128, 4], F32, kind="ExternalInput")
    out_d = nc.dram_tensor("out", [TOK, D], F32, kind="ExternalOutput")

    # DRAM scratch for the T2e diagonal round trip, one per (b, lc), all heads
    t2scr = [[nc.dram_tensor(f"t2_{b}_{lc}", [128, T2WIDE], BF)
              for lc in range(4)] for b in range(BL)]

    with tile.TileContext(nc) as tc:
        import contextlib
        ctx = contextlib.ExitStack()
        with ctx:
            singles = ctx.enter_context(tc.tile_pool(name="singles", bufs=1))
            xload = ctx.enter_context(tc.tile_pool(name="xload", bufs=2))
            projs = ctx.enter_context(tc.tile_pool(name="projs", bufs=1))
            mid = ctx.enter_context(tc.tile_pool(name="mid", bufs=1))
            wide = ctx.enter_context(tc.tile_pool(name="wide", bufs=2))
            expp = ctx.enter_context(tc.tile_pool(name="expp", bufs=1))
            small = ctx.enter_context(tc.tile_pool(name="small", bufs=2))
            trans = ctx.enter_context(tc.tile_pool(name="trans", bufs=4))
            pwide = ctx.enter_context(tc.tile_pool(name="pwide", bufs=2, space="PSUM"))
            ps512 = ctx.enter_context(tc.tile_pool(name="ps512", bufs=4, space="PSUM"))

            # --- constants ---
            w_sb = {}
            for name in cw:
                t = singles.tile([128, 4, D], BF, tag=name)
                nc.sync.dma_start(t[:], cw[name][:, :, :])
                w_sb[name] = t
            relx = singles.tile([128, 1032], BF, tag="relx")
            nc.sync.dma_start(relx[:], relx_d[:, :])
            ident = singles.tile([128, 128], BF, tag="ident")
            nc.sync.dma_start(ident[:], ident_d[:, :])
            ones_bf = singles.tile([128, 1], BF, tag="ones_bf")
            nc.vector.memset(ones_bf[:], 1.0)
            ones_bfr = singles.tile([1, 128], BF, tag="ones_bfr")
            nc.vector.memset(ones_bfr[:], 1.0)
            pb_sb = {}
            for name, t in pbias_d.items():
                s = singles.tile([128, 4], F32, tag=name)
                nc.sync.dma_start(s[:], t[:, :])
                pb_sb[name] = s
            fb_sb = {}
            for name, t in fbias_d.items():
                s = singles.tile([128, D], F32, tag=name)
                nc.sync.dma_start(s[:], bass.AP(t, 0, [[0, 128], [1, D]]))
                fb_sb[name] = s

            evac_ctr = [0]

            def evac_copy(dst, src, bias_col=None, ratio=2):
                """PSUM->SBUF copy (+ optional per-partition bias), alternating
                DVE/ACT at ratio:1."""
                if bias_col is not None:
                    nc.vector.tensor_scalar_add(dst, src, bias_col)
                    return
                if evac_ctr[0] % (ratio + 1) < ratio:
                    nc.vector.tensor_copy(dst, src)
                else:
                    nc.scalar.activation(dst, src, AF.Copy)
                evac_ctr[0] += 1

            for rep in range(reps):
              for b in range(BL):
                tok0 = b * S

                # ---- inputs arrive host-transposed: xT [128(din%128), 4(ic), tok]
                xT = {}
                for key, xd in (("q", x_q), ("k", x_k), ("v", x_v)):
                    xT_t = xload.tile([128, 4, S], BF, tag=f"xT_{key}")
                    nc.sync.dma_start(xT_t[:], xd[:, :, tok0:tok0 + S])
                    xT[key] = xT_t

                # ---- projections
                def projT(dst, wname, xkey, bias_name):
                    # dst [128(dout%128), 4(oc), 512(l)]  (T-layout out)
                    for oc in range(4):
                        ps = ps512.tile([128, S], F32, tag="mm512")
                        for ic in range(4):
                            nc.tensor.matmul(ps[:], w_sb[wname][:, ic, oc * 128:(oc + 1) * 128],
                                             xT[xkey][:, ic, :], start=(ic == 0), stop=(ic == 3))
                        bias_col = pb_sb[bias_name][:, oc:oc + 1] if bias_name in pb_sb else None
                        evac_copy(dst[:, oc, :], ps[:], bias_col)

                def projN(dst, wname, xkey):
                    # dst [128(tok%128), 4(tc), 512(dout)]  (natural out)
                    # (bias along free dim is deferred to the attention output)
                    for tck in range(4):
                        ps = ps512.tile([128, D], F32, tag="mm512")
                        for ic in range(4):
                            nc.tensor.matmul(ps[:], xT[xkey][:, ic, tck * 128:(tck + 1) * 128],
                                             w_sb[wname][:, ic, :], start=(ic == 0), stop=(ic == 3))
                        evac_copy(dst[:, tck, :], ps[:])

                qT8 = projs.tile([128, 4, S], BF, tag="qT8")
                kT = projs.tile([128, 4, S], BF, tag="kT")
                gqT = projs.tile([128, 4, S], BF, tag="gqT")
                gkT = projs.tile([128, 4, S], BF, tag="gkT")
                v_n = projs.tile([128, 4, D], BF, tag="v_n")
                vg_n = projs.tile([128, 4, D], BF, tag="vg_n")
                projT(qT8, "cqT", "q", "bqT")
                projT(kT, "ckT", "k", "bkT")
                projT(gqT, "cgqT", "q", "bgqT")
                projT(gkT, "cgkT", "k", "bgkT")
                projN(v_n, "cvT", "v")
                projN(vg_n, "cgvT", "v")

                localT = mid.tile([128, 4, S], BF, tag="localT")
                expT = expp.tile([128, H, 4, S], BF, tag="expT")
                sums = small.tile([128, 32], F32, tag="sums")
                rec = small.tile([128, 32], F32, tag="rec")

                # ---- local attention ----
                for lc in range(4):
                    u0 = 385 - lc * 128
                    # T2e for all 8 heads; even/odd row-tiled pairs
                    t2w = wide.tile([128, H, T2W], BF, tag="t2w")
                    for oc in range(4):
                        qsA = qT8[0:64, oc, lc * 128:(lc + 1) * 128]
                        qsB = qT8[64:128, oc, lc * 128:(lc + 1) * 128]
                        pwA = pwide.tile([128, 1024], F32, tag="pw")
                        pwB = pwide.tile([128, 1024], F32, tag="pw")
                        nc.tensor.matmul(pwA[:, 0:512], qsA, relx[0:64, u0:u0 + 512], start=True, stop=True)
                        nc.tensor.matmul(pwB[:, 0:512], qsB, relx[64:128, u0:u0 + 512], start=True, stop=True)
                        nc.tensor.matmul(pwA[:, 512:640], qsA, relx[0:64, u0 + 512:u0 + 640], start=True, stop=True)
                        nc.tensor.matmul(pwB[:, 512:640], qsB, relx[64:128, u0 + 512:u0 + 640], start=True, stop=True)
                        nc.vector.tensor_copy(t2w[:, 2 * oc, :], pwA[:, 0:T2W])
                        nc.vector.tensor_copy(t2w[:, 2 * oc + 1, :], pwB[:, 0:T2W])
                    nc.sync.dma_start(t2scr[b][lc][:, :], t2w[:])
                    # diagonal (skew) read: biasw[l, h, r] = T2e[l, h, r - l + 127]
                    biasw = wide.tile([128, H, S], BF, tag="biasw")
                    nc.sync.dma_start(
                        biasw[:],
                        bass.AP(t2scr[b][lc], 127, [[T2WIDE - 1, 128], [T2W, H], [1, S]]))

                    # scores + exp for all 8 heads of this l-chunk
                    expn = wide.tile([128, H, S], BF, tag="expn")
                    for oc in range(4):
                        qsA = qT8[0:64, oc, lc * 128:(lc + 1) * 128]
                        qsB = qT8[64:128, oc, lc * 128:(lc + 1) * 128]
                        psA = ps512.tile([128, S], F32, tag="mm512")
                        psB = ps512.tile([128, S], F32, tag="mm512")
                        nc.tensor.matmul(psA[:], qsA, kT[0:64, oc, :], start=True, stop=False)
                        nc.tensor.matmul(psB[:], qsB, kT[64:128, oc, :], start=True, stop=False)
                        nc.tensor.matmul(psA[:], ident[:], biasw[:, 2 * oc, :], start=False, stop=True)
                        nc.tensor.matmul(psB[:], ident[:], biasw[:, 2 * oc + 1, :], start=False, stop=True)
                        nc.scalar.activation(expn[:, 2 * oc, :], psA[:], AF.Exp,
                                             accum_out=sums[:, lc * 8 + 2 * oc:lc * 8 + 2 * oc + 1])
                        nc.scalar.activation(expn[:, 2 * oc + 1, :], psB[:], AF.Exp,
                                             accum_out=sums[:, lc * 8 + 2 * oc + 1:lc * 8 + 2 * oc + 2])
                    # batched reciprocal + per-head 0.7/sums normalize on Pool
                    nc.vector.reciprocal(rec[:, lc * 8:(lc + 1) * 8], sums[:, lc * 8:(lc + 1) * 8])
                    for h in range(H):
                        nc.gpsimd.tensor_scalar(expn[:, h, :], expn[:, h, :],
                                                rec[:, lc * 8 + h:lc * 8 + h + 1], 0.7,
                                                ALU.mult, ALU.mult)
                    # one XBAR transpose: expT[r%128, h, rc, l-chunk] <- expn[l, h, r]
                    nc.sync.dma_start(expT[:, :, :, lc * 128:(lc + 1) * 128], expn[:, :, :],
                                      transpose=True)

                # AV, col-tiled per head pair (M=64 halves of the array)
                for oc in range(4):
                    pav = ps512.tile([128, S], F32, tag="mm512")
                    for rc in range(4):
                        nc.tensor.matmul(pav[0:64, :], v_n[:, rc, oc * 128:oc * 128 + 64],
                                         expT[:, 2 * oc, rc, :], start=(rc == 0), stop=(rc == 3))
                        nc.tensor.matmul(pav[64:128, :], v_n[:, rc, oc * 128 + 64:(oc + 1) * 128],
                                         expT[:, 2 * oc + 1, rc, :], start=(rc == 0), stop=(rc == 3))
                    if "bvT" in pb_sb:
                        # deferred v bias (scaled 0.7 on host): out = AV*0.7/sums + 0.7*bv
                        nc.vector.tensor_scalar_add(localT[:, oc, :], pav[:],
                                                    pb_sb["bvT"][:, oc:oc + 1])
                    else:
                        nc.scalar.activation(localT[:, oc, :], pav[:], AF.Copy)

                # ---- global attention ----
                gT = mid.tile([128, 4, S], BF, tag="gT")
                for hg in range(HG):
                    expTg = expp.tile([128, 4, S], BF, tag="expTg")
                    for rc in range(4):
                        ps = ps512.tile([128, S], F32, tag="mm512")
                        nc.tensor.matmul(ps[:], gkT[:, hg, rc * 128:(rc + 1) * 128],
                                         gqT[:, hg, :], start=True, stop=True)
                        nc.scalar.activation(expTg[:, rc, :], ps[:], AF.Exp)
                    psum = ps512.tile([128, S], F32, tag="mm512")
                    for rc in range(4):
                        nc.tensor.matmul(psum[0:1, :], ones_bf[:], expTg[:, rc, :],
                                         start=(rc == 0), stop=(rc == 3))
                    recip_g = trans.tile([1, S], BF, tag="recip_g")
                    with nc.allow_low_precision(reason="bf16 recip feeds bf16 bcast matmul; error cancels with bf16 exp sums"):
                        nc.vector.reciprocal(recip_g[:], psum[0:1, :])
                    pbc = ps512.tile([128, S], F32, tag="mm512")
                    nc.tensor.matmul(pbc[:], ones_bfr[:], recip_g[:], start=True, stop=True)
                    recipB = trans.tile([128, S], F32, tag="recipB")
                    nc.scalar.activation(recipB[:], pbc[:], AF.Copy)
                    pgT = ps512.tile([128, S], F32, tag="mm512")
                    for rc in range(4):
                        nc.tensor.matmul(pgT[:], vg_n[:, rc, hg * 128:(hg + 1) * 128],
                                         expTg[:, rc, :], start=(rc == 0), stop=(rc == 3))
                    nc.vector.tensor_tensor(gT[:, hg, :], pgT[:], recipB[:], ALU.mult)
                    if "bgvb" in fb_sb:
                        nc.vector.tensor_scalar_add(gT[:, hg, :], gT[:, hg, :],
                                                    fb_sb["bgvb"][hg * 128:(hg + 1) * 128, 0:1])

                # ---- g_out proj (T-layout, x0.3 folded) + combine with localT ----
                combT = mid.tile([128, 4, S], BF, tag="combT")
                for oc in range(4):
                    ps = ps512.tile([128, S], F32, tag="mm512")
                    for ic in range(4):
                        nc.tensor.matmul(ps[:], w_sb["cgoT"][:, ic, oc * 128:(oc + 1) * 128],
                                         gT[:, ic, :], start=(ic == 0), stop=(ic == 3))
                    if "bgoT" in pb_sb:
                        nc.vector.tensor_scalar_add(ps[:], ps[:], pb_sb["bgoT"][:, oc:oc + 1])
                    nc.vector.tensor_tensor(combT[:, oc, :], ps[:], localT[:, oc, :], ALU.add)

                # ---- final wo ----
                for tck in range(4):
                    ps = ps512.tile([128, D], F32, tag="mm512")
                    for ic in range(4):
                        nc.tensor.matmul(ps[:], combT[:, ic, tck * 128:(tck + 1) * 128],
                                         w_sb["cwoT"][:, ic, :], start=(ic == 0), stop=(ic == 3))
                    osb = trans.tile([128, D], F32, tag="osb")
                    if "bob" in fb_sb:
                        nc.vector.tensor_tensor(osb[:], ps[:], fb_sb["bob"][:], ALU.add)
                    else:
                        nc.scalar.activation(osb[:], ps[:], AF.Copy)
                    nc.sync.dma_start(out_d[tok0 + tck * 128: tok0 + (tck + 1) * 128, :], osb[:])

    _split_wide_waits(nc)
    return nc


def _arr_T(w, scale=1.0):
    """torch Linear weight [dout, din] -> lhsT const [128, 4, dout] bf16 with
    [p, ic, o] = w[o, ic*128+p]*scale."""
    wt = (w.T.astype(np.float64) * scale).astype(np.float32)
    return np.ascontiguousarray(
        wt.reshape(4, 128, wt.shape[1]).transpose(1, 0, 2)).astype(ml_dtypes.bfloat16)


def _arr_bias(bvec, scale=1.0):
    """bias [512] -> [128, 4] f32 with [p, oc] = b[oc*128+p]*scale."""
    return np.ascontiguousarray(
        (np.asarray(bvec) * scale).astype(np.float32).reshape(4, 128).T)


def kernel(query, key, value, wq, bq, wk, bk, wv, bv, wo, bo, rel_k,
           g_in_w, g_in_b, g_out_w, g_out_b):
    query = np.asarray(query, dtype=np.float32)
    key = np.asarray(key, dtype=np.float32)
    value = np.asarray(value, dtype=np.float32)

    sg = 1.0 / np.sqrt(np.float32(HDG))
    consts = {
        "cqT": _arr_T(np.asarray(wq), 0.125),
        "ckT": _arr_T(np.asarray(wk)),
        "cvT": _arr_T(np.asarray(wv)),
        "cgqT": _arr_T(np.asarray(g_in_w)[:D], sg),
        "cgkT": _arr_T(np.asarray(g_in_w)[D:2 * D]),
        "cgvT": _arr_T(np.asarray(g_in_w)[2 * D:]),
        "cgoT": _arr_T(np.asarray(g_out_w), 0.3),
        "cwoT": _arr_T(np.asarray(wo)),
        "ident": np.eye(128, dtype=ml_dtypes.bfloat16),
    }
    # rel_ext [128, 1032]: rows 0-63 and 64-127 both hold
    # [d, u] = 8*rel_k[clip(u-256, 0, 512), d] (duplicated so lhsT/rhs base
    # partitions match for odd heads)
    idx = np.clip(np.arange(EXTW) - 256, 0, 2 * MAXREL)
    tbl = (8.0 * np.asarray(rel_k, dtype=np.float32)[idx]).T  # [64, 1025]
    relx = np.zeros((128, 1032), dtype=ml_dtypes.bfloat16)
    relx[:HD, :EXTW] = tbl.astype(ml_dtypes.bfloat16)
    relx[HD:2 * HD, :EXTW] = tbl.astype(ml_dtypes.bfloat16)
    consts["relx"] = relx

    bq, bk, bv = np.asarray(bq), np.asarray(bk), np.asarray(bv)
    bo, g_in_b, g_out_b = np.asarray(bo), np.asarray(g_in_b), np.asarray(g_out_b)
    flags = (bool(np.any(bq)), bool(np.any(bk)), bool(np.any(bv)),
             bool(np.any(g_in_b[:D])), bool(np.any(g_in_b[D:2 * D])),
             bool(np.any(g_in_b[2 * D:])), bool(np.any(g_out_b)), bool(np.any(bo)))
    if flags[0]:
        consts["bqT"] = _arr_bias(bq, 0.125)
    if flags[1]:
        consts["bkT"] = _arr_bias(bk)
    if flags[2]:
        consts["bvT"] = _arr_bias(bv, 0.7)   # deferred past softmax, 0.7 fold
    if flags[3]:
        consts["bgqT"] = _arr_bias(g_in_b[:D], sg)
    if flags[4]:
        consts["bgkT"] = _arr_bias(g_in_b[D:2 * D])
    if flags[5]:
        consts["bgvb"] = g_in_b[2 * D:].astype(np.float32)
    if flags[6]:
        consts["bgoT"] = _arr_bias(g_out_b, 0.3)
    if flags[7]:
        consts["bob"] = bo.astype(np.float32)

    if flags not in _cache:
        _cache[flags] = _build(flags)
    nc = _cache[flags]

    def _xT(x_core):
        # [BL, S, D] f32 -> [128, 4(ic), TOK] bf16 with [p, ic, t] = x[t, ic*128+p]
        xt = x_core.reshape(TOK, D).T.astype(ml_dtypes.bfloat16)   # [D, TOK]
        return np.ascontiguousarray(xt.reshape(4, 128, TOK).transpose(1, 0, 2))

    in_maps = []
    for c in range(NCORES):
        m = dict(consts)
        m["query"] = _xT(query[c * BL:(c + 1) * BL])
        m["key"] = _xT(key[c * BL:(c + 1) * BL])
        m["value"] = _xT(value[c * BL:(c + 1) * BL])
        in_maps.append(m)

    res = run_bass_kernel_spmd(nc, in_maps, core_ids=list(range(NCORES)))
    global LAST_RESULT
    LAST_RESULT = res
    out = np.concatenate(
        [r["out"].reshape(BL, S, D) for r in res.results], axis=0)
    return out.astype(np.float32)


LAST_RESULT = None


# revision 6
# speedup vs baseline: 1.0894x; 1.0708x over previous
"""Trainium2 Bass kernel for EnhancedTransformerNetwork (local rel-pos attention + global MHA).

Sharding: data-parallel over batch. 8 cores x 4 batches each; weights replicated.

Per-core algorithm (tokens = 4*512 = 2048, all matmuls bf16, PSUM f32):
  - inputs arrive host-transposed bf16 [din, tokens]
  - local:  qT8 = (wq/8)^T x^T (T-layout), kT (T), v natural.
            Per (batch, l-chunk): rel bias T2e for all 8 heads, with even/odd
            head matmuls row-tiled on array halves (K=64), evac'd into one
            wide tile, ONE DRAM write + ONE 3D diagonal-AP read (clip baked
            into the host rel_ext table) -> biasw[l, h, r].
            scores psum = qk (row-tiled pair, start) + ident-matmul(bias);
            exp on ACT with accum_out row sums; batched reciprocal per chunk;
            0.7/sums normalization on Pool; ONE DMA XBAR transpose per
            (batch, l-chunk) turns all 8 heads' exp [l, r] into expT [r, l];
            AV col-tiled per head pair (M=64 halves) -> localT [hd, l].
  - global: scoresT[r,l] = gkT^T gqT (K=128), exp; sums via ones-matmul;
            recip broadcast via fp32 K=1 PE outer product; AV with vg as lhsT
            -> gT [hdg, l] T-layout; g_out proj in T-layout (0.3 folded).
  - combT = localT + gprojT, final wo matmul, DMA out.
"""

import numpy as np
import ml_dtypes

import concourse.bass as bass
import concourse.mybir as mybir
import concourse.tile as tile
from concourse.bass_utils import run_bass_kernel_spmd

BF = mybir.dt.bfloat16
F32 = mybir.dt.float32
AF = mybir.ActivationFunctionType
ALU = mybir.AluOpType

D = 512
H = 8
HD = 64
HG = 4
HDG = 128
MAXREL = 256
S = 512
B = 32
NCORES = 8
BL = B // NCORES          # batches per core
TOK = BL * S              # tokens per core
EXTW = 1025               # rel_ext width: u' in [0, 1024], clip baked in
T2W = 640                 # per-chunk T2e width (639 used, padded to 640)
T2WIDE = H * T2W          # 5120: all 8 heads' T2e rows side by side

_cache = {}


def _split_wide_waits(nc, max_waits=1):
    """This walrus build supports only one sem wait per instruction; Tile's
    final drain carries one wait per logical proc. Hoist excess waits onto
    preceding same-engine NOPs (same-engine program order keeps semantics)."""
    f = nc.m.functions[0]
    created = {}
    for bb in f.blocks:
        for inst in bb.instructions:
            si = inst.sync_info
            if si is not None and len(si.on_wait) > max_waits:
                waits = list(si.on_wait)
                head, tail = waits[:-max_waits], waits[-max_waits:]
                nops = []
                for i in range(0, len(head), max_waits):
                    nop = nc.engines[inst.engine].nop(nofuse=True)
                    nop.ins.sync_info = mybir.SyncInfo(
                        on_wait=head[i:i + max_waits], on_update=[])
                    nops.append(nop.ins)
                si.on_wait = tail
                created[inst.name] = nops
    if not created:
        return
    names = {n.name for nops in created.values() for n in nops}
    for bb in f.blocks:
        out = []
        for inst in bb.instructions:
            if inst.name in names:
                continue
            if inst.name in created:
                out.extend(created[inst.name])
            out.append(inst)
        bb.instructions = out


def _build(flags, reps=1):
    """flags: (has_bq, has_bk, has_bv, has_bgq, has_bgk, has_bgv, has_bgo, has_bo)
    reps>1 replicates the whole computation on device (benchmarking only)."""
    has_bq, has_bk, has_bv, has_bgq, has_bgk, has_bgv, has_bgo, has_bo = flags
    nc = bass.Bass("TRN2", target_bir_lowering=False, debug=False,
                   num_devices=NCORES)

    # --- I/O ---
    x_q = nc.dram_tensor("query", [128, 4, TOK], BF, kind="ExternalInput")
    x_k = nc.dram_tensor("key", [128, 4, TOK], BF, kind="ExternalInput")
    x_v = nc.dram_tensor("value", [128, 4, TOK], BF, kind="ExternalInput")
    cw = {}
    for name in ["cqT", "ckT", "cvT", "cgqT", "cgkT", "cgvT", "cgoT", "cwoT"]:
        cw[name] = nc.dram_tensor(name, [128, 4, D], BF, kind="ExternalInput")
    relx_d = nc.dram_tensor("relx", [128, 1032], BF, kind="ExternalInput")
    ident_d = nc.dram_tensor("ident", [128, 128], BF, kind="ExternalInput")
    pbias_d = {}
    for name, has in [("bqT", has_bq), ("bkT", has_bk), ("bvT", has_bv),
                      ("bgqT", has_bgq), ("bgkT", has_bgk)]:
        if has:
            pbias_d[name] = nc.dram_tensor(name, [128, 4], F32, kind="ExternalInput")
    fbias_d = {}
    for name, has in [("bgvb", has_bgv), ("bob", has_bo)]:
        if has:
            fbias_d[name] = nc.dram_tensor(name, [D], F32, kind="ExternalInput")
    if has_bgo:
        pbias_d["bgoT"] = nc.dram_tensor("bgoT", [128, 4], F32, kind="ExternalInput")
    out_d = nc.dram_tensor("out", [TOK, D], F32, kind="ExternalOutput")

    # DRAM scratch for the T2e diagonal round trip, one per (b, lc), all heads
    t2scr = [[nc.dram_tensor(f"t2_{b}_{lc}", [128, T2WIDE], BF)
              for lc in range(4)] for b in range(BL)]

    with tile.TileContext(nc) as tc:
        import contextlib
        ctx = contextlib.ExitStack()
        with ctx:
            singles = ctx.enter_context(tc.tile_pool(name="singles", bufs=1))
            xload = ctx.enter_context(tc.tile_pool(name="xload", bufs=2))
            projs = ctx.enter_context(tc.tile_pool(name="projs", bufs=1))
            mid = ctx.enter_context(tc.tile_pool(name="mid", bufs=1))
            wide = ctx.enter_context(tc.tile_pool(name="wide", bufs=2))
            biaswp = ctx.enter_context(tc.tile_pool(name="biaswp", bufs=3))
            expp = ctx.enter_context(tc.tile_pool(name="expp", bufs=1))
            small = ctx.enter_context(tc.tile_pool(name="small", bufs=2))
            trans = ctx.enter_context(tc.tile_pool(name="trans", bufs=2))
            pwide = ctx.enter_context(tc.tile_pool(name="pwide", bufs=2, space="PSUM"))
            ps512 = ctx.enter_context(tc.tile_pool(name="ps512", bufs=4, space="PSUM"))

            # --- constants ---
            w_sb = {}
            for name in cw:
                t = singles.tile([128, 4, D], BF, tag=name)
                nc.sync.dma_start(t[:], cw[name][:, :, :])
                w_sb[name] = t
            relx = singles.tile([128, 1032], BF, tag="relx")
            nc.sync.dma_start(relx[:], relx_d[:, :])
            ident = singles.tile([128, 128], BF, tag="ident")
            nc.sync.dma_start(ident[:], ident_d[:, :])
            ones_bf = singles.tile([128, 1], BF, tag="ones_bf")
            nc.vector.memset(ones_bf[:], 1.0)
            ones_bfr = singles.tile([1, 128], BF, tag="ones_bfr")
            nc.vector.memset(ones_bfr[:], 1.0)
            pb_sb = {}
            for name, t in pbias_d.items():
                s = singles.tile([128, 4], F32, tag=name)
                nc.sync.dma_start(s[:], t[:, :])
                pb_sb[name] = s
            fb_sb = {}
            for name, t in fbias_d.items():
                s = singles.tile([128, D], F32, tag=name)
                nc.sync.dma_start(s[:], bass.AP(t, 0, [[0, 128], [1, D]]))
                fb_sb[name] = s

            evac_ctr = [0]

            def evac_copy(dst, src, bias_col=None, ratio=2):
                """PSUM->SBUF copy (+ optional per-partition bias), alternating
                DVE/ACT at ratio:1."""
                if bias_col is not None:
                    nc.vector.tensor_scalar_add(dst, src, bias_col)
                    return
                if evac_ctr[0] % (ratio + 1) < ratio:
                    nc.vector.tensor_copy(dst, src)
                else:
                    nc.scalar.activation(dst, src, AF.Copy)
                evac_ctr[0] += 1

            for rep in range(reps):
              for b in range(BL):
                tok0 = b * S

                # ---- inputs arrive host-transposed: xT [128(din%128), 4(ic), tok]
                xT = {}
                for key, xd in (("q", x_q), ("k", x_k), ("v", x_v)):
                    xT_t = xload.tile([128, 4, S], BF, tag=f"xT_{key}")
                    nc.sync.dma_start(xT_t[:], xd[:, :, tok0:tok0 + S])
                    xT[key] = xT_t

                # ---- projections
                def projT(dst, wname, xkey, bias_name):
                    # dst [128(dout%128), 4(oc), 512(l)]  (T-layout out)
                    for oc in range(4):
                        ps = ps512.tile([128, S], F32, tag="mm512")
                        for ic in range(4):
                            nc.tensor.matmul(ps[:], w_sb[wname][:, ic, oc * 128:(oc + 1) * 128],
                                             xT[xkey][:, ic, :], start=(ic == 0), stop=(ic == 3))
                        bias_col = pb_sb[bias_name][:, oc:oc + 1] if bias_name in pb_sb else None
                        evac_copy(dst[:, oc, :], ps[:], bias_col)

                def projN(dst, wname, xkey):
                    # dst [128(tok%128), 4(tc), 512(dout)]  (natural out)
                    # (bias along free dim is deferred to the attention output)
                    for tck in range(4):
                        ps = ps512.tile([128, D], F32, tag="mm512")
                        for ic in range(4):
                            nc.tensor.matmul(ps[:], xT[xkey][:, ic, tck * 128:(tck + 1) * 128],
                                             w_sb[wname][:, ic, :], start=(ic == 0), stop=(ic == 3))
                        evac_copy(dst[:, tck, :], ps[:])

                qT8 = projs.tile([128, 4, S], BF, tag="qT8")
                kT = projs.tile([128, 4, S], BF, tag="kT")
                gqT = projs.tile([128, 4, S], BF, tag="gqT")
                gkT = projs.tile([128, 4, S], BF, tag="gkT")
                v_n = projs.tile([128, 4, D], BF, tag="v_n")
                vg_n = projs.tile([128, 4, D], BF, tag="vg_n")
                projT(qT8, "cqT", "q", "bqT")
                projT(kT, "ckT", "k", "bkT")
                projT(gqT, "cgqT", "q", "bgqT")
                projT(gkT, "cgkT", "k", "bgkT")
                projN(v_n, "cvT", "v")
                projN(vg_n, "cgvT", "v")

                localT = mid.tile([128, 4, S], BF, tag="localT")
                expT = expp.tile([128, H, 4, S], BF, tag="expT")
                sums = small.tile([128, 32], F32, tag="sums")
                rec = small.tile([128, 32], F32, tag="rec")

                # ---- local phase A: T2e + DRAM skew round trips, all 4 l-chunks ----
                # (emitted first so all 4 round trips pipeline; biaswp bufs=4)
                biasw = []
                for lc in range(4):
                    u0 = 385 - lc * 128
                    # T2e for all 8 heads; even/odd row-tiled pairs
                    t2w = wide.tile([128, H, T2W], BF, tag="t2w")
                    for oc in range(4):
                        qsA = qT8[0:64, oc, lc * 128:(lc + 1) * 128]
                        qsB = qT8[64:128, oc, lc * 128:(lc + 1) * 128]
                        pwA = pwide.tile([128, 1024], F32, tag="pw")
                        pwB = pwide.tile([128, 1024], F32, tag="pw")
                        nc.tensor.matmul(pwA[:, 0:512], qsA, relx[0:64, u0:u0 + 512], start=True, stop=True)
                        nc.tensor.matmul(pwB[:, 0:512], qsB, relx[64:128, u0:u0 + 512], start=True, stop=True)
                        nc.tensor.matmul(pwA[:, 512:640], qsA, relx[0:64, u0 + 512:u0 + 640], start=True, stop=True)
                        nc.tensor.matmul(pwB[:, 512:640], qsB, relx[64:128, u0 + 512:u0 + 640], start=True, stop=True)
                        nc.vector.tensor_copy(t2w[:, 2 * oc, :], pwA[:, 0:T2W])
                        nc.vector.tensor_copy(t2w[:, 2 * oc + 1, :], pwB[:, 0:T2W])
                    nc.sync.dma_start(t2scr[b][lc][:, :], t2w[:])
                    # diagonal (skew) read: biasw[l, h, r] = T2e[l, h, r - l + 127]
                    bw = biaswp.tile([128, H, S], BF, tag="biasw")
                    nc.sync.dma_start(
                        bw[:],
                        bass.AP(t2scr[b][lc], 127, [[T2WIDE - 1, 128], [T2W, H], [1, S]]))
                    biasw.append(bw)

                # ---- global attention (emitted here: its PE work fills the
                # local pipeline's DMA latency) ----
                gT = mid.tile([128, 4, S], BF, tag="gT")
                for hg in range(HG):
                    expTg = trans.tile([128, 4, S], BF, tag="expTg")
                    for rc in range(4):
                        ps = ps512.tile([128, S], F32, tag="mm512")
                        nc.tensor.matmul(ps[:], gkT[:, hg, rc * 128:(rc + 1) * 128],
                                         gqT[:, hg, :], start=True, stop=True)
                        nc.scalar.activation(expTg[:, rc, :], ps[:], AF.Exp)
                    psum = ps512.tile([128, S], F32, tag="mm512")
                    for rc in range(4):
                        nc.tensor.matmul(psum[0:1, :], ones_bf[:], expTg[:, rc, :],
                                         start=(rc == 0), stop=(rc == 3))
                    recip_g = trans.tile([1, S], BF, tag="recip_g")
                    with nc.allow_low_precision(reason="bf16 recip feeds bf16 bcast matmul; error cancels with bf16 exp sums"):
                        nc.vector.reciprocal(recip_g[:], psum[0:1, :])
                    pbc = ps512.tile([128, S], F32, tag="mm512")
                    nc.tensor.matmul(pbc[:], ones_bfr[:], recip_g[:], start=True, stop=True)
                    recipB = trans.tile([128, S], F32, tag="recipB")
                    nc.scalar.activation(recipB[:], pbc[:], AF.Copy)
                    pgT = ps512.tile([128, S], F32, tag="mm512")
                    for rc in range(4):
                        nc.tensor.matmul(pgT[:], vg_n[:, rc, hg * 128:(hg + 1) * 128],
                                         expTg[:, rc, :], start=(rc == 0), stop=(rc == 3))
                    nc.vector.tensor_tensor(gT[:, hg, :], pgT[:], recipB[:], ALU.mult)
                    if "bgvb" in fb_sb:
                        nc.vector.tensor_scalar_add(gT[:, hg, :], gT[:, hg, :],
                                                    fb_sb["bgvb"][hg * 128:(hg + 1) * 128, 0:1])

                # ---- local phase B: scores + exp + normalize + transpose ----
                for lc in range(4):
                    expn = wide.tile([128, H, S], BF, tag="expn")
                    for oc in range(4):
                        qsA = qT8[0:64, oc, lc * 128:(lc + 1) * 128]
                        qsB = qT8[64:128, oc, lc * 128:(lc + 1) * 128]
                        psA = ps512.tile([128, S], F32, tag="mm512")
                        psB = ps512.tile([128, S], F32, tag="mm512")
                        nc.tensor.matmul(psA[:], qsA, kT[0:64, oc, :], start=True, stop=False)
                        nc.tensor.matmul(psB[:], qsB, kT[64:128, oc, :], start=True, stop=False)
                        nc.tensor.matmul(psA[:], ident[:], biasw[lc][:, 2 * oc, :], start=False, stop=True)
                        nc.tensor.matmul(psB[:], ident[:], biasw[lc][:, 2 * oc + 1, :], start=False, stop=True)
                        nc.scalar.activation(expn[:, 2 * oc, :], psA[:], AF.Exp,
                                             accum_out=sums[:, lc * 8 + 2 * oc:lc * 8 + 2 * oc + 1])
                        nc.scalar.activation(expn[:, 2 * oc + 1, :], psB[:], AF.Exp,
                                             accum_out=sums[:, lc * 8 + 2 * oc + 1:lc * 8 + 2 * oc + 2])
                    # batched reciprocal + per-head 0.7/sums normalize on Pool
                    nc.vector.reciprocal(rec[:, lc * 8:(lc + 1) * 8], sums[:, lc * 8:(lc + 1) * 8])
                    for h in range(H):
                        nc.gpsimd.tensor_scalar(expn[:, h, :], expn[:, h, :],
                                                rec[:, lc * 8 + h:lc * 8 + h + 1], 0.7,
                                                ALU.mult, ALU.mult)
                    # one XBAR transpose: expT[r%128, h, rc, l-chunk] <- expn[l, h, r]
                    nc.sync.dma_start(expT[:, :, :, lc * 128:(lc + 1) * 128], expn[:, :, :],
                                      transpose=True)

                # ---- local phase C: AV, col-tiled per head pair (M=64 halves) ----
                for oc in range(4):
                    pav = ps512.tile([128, S], F32, tag="mm512")
                    for rc in range(4):
                        nc.tensor.matmul(pav[0:64, :], v_n[:, rc, oc * 128:oc * 128 + 64],
                                         expT[:, 2 * oc, rc, :], start=(rc == 0), stop=(rc == 3))
                        nc.tensor.matmul(pav[64:128, :], v_n[:, rc, oc * 128 + 64:(oc + 1) * 128],
                                         expT[:, 2 * oc + 1, rc, :], start=(rc == 0), stop=(rc == 3))
                    if "bvT" in pb_sb:
                        # deferred v bias (scaled 0.7 on host): out = AV*0.7/sums + 0.7*bv
                        nc.vector.tensor_scalar_add(localT[:, oc, :], pav[:],
                                                    pb_sb["bvT"][:, oc:oc + 1])
                    else:
                        nc.vector.tensor_copy(localT[:, oc, :], pav[:])

                # ---- g_out proj (T-layout, x0.3 folded) + combine with localT ----
                combT = mid.tile([128, 4, S], BF, tag="combT")
                for oc in range(4):
                    ps = ps512.tile([128, S], F32, tag="mm512")
                    for ic in range(4):
                        nc.tensor.matmul(ps[:], w_sb["cgoT"][:, ic, oc * 128:(oc + 1) * 128],
                                         gT[:, ic, :], start=(ic == 0), stop=(ic == 3))
                    if "bgoT" in pb_sb:
                        nc.vector.tensor_scalar_add(ps[:], ps[:], pb_sb["bgoT"][:, oc:oc + 1])
                    nc.vector.tensor_tensor(combT[:, oc, :], ps[:], localT[:, oc, :], ALU.add)

                # ---- final wo ----
                for tck in range(4):
                    ps = ps512.tile([128, D], F32, tag="mm512")
                    for ic in range(4):
                        nc.tensor.matmul(ps[:], combT[:, ic, tck * 128:(tck + 1) * 128],
                                         w_sb["cwoT"][:, ic, :], start=(ic == 0), stop=(ic == 3))
                    osb = trans.tile([128, D], F32, tag="osb")
                    if "bob" in fb_sb:
                        nc.vector.tensor_tensor(osb[:], ps[:], fb_sb["bob"][:], ALU.add)
                    else:
                        nc.scalar.activation(osb[:], ps[:], AF.Copy)
                    nc.sync.dma_start(out_d[tok0 + tck * 128: tok0 + (tck + 1) * 128, :], osb[:])

    _split_wide_waits(nc)
    return nc


def _arr_T(w, scale=1.0):
    """torch Linear weight [dout, din] -> lhsT const [128, 4, dout] bf16 with
    [p, ic, o] = w[o, ic*128+p]*scale."""
    wt = (w.T.astype(np.float64) * scale).astype(np.float32)
    return np.ascontiguousarray(
        wt.reshape(4, 128, wt.shape[1]).transpose(1, 0, 2)).astype(ml_dtypes.bfloat16)


def _arr_bias(bvec, scale=1.0):
    """bias [512] -> [128, 4] f32 with [p, oc] = b[oc*128+p]*scale."""
    return np.ascontiguousarray(
        (np.asarray(bvec) * scale).astype(np.float32).reshape(4, 128).T)


def kernel(query, key, value, wq, bq, wk, bk, wv, bv, wo, bo, rel_k,
           g_in_w, g_in_b, g_out_w, g_out_b):
    query = np.asarray(query, dtype=np.float32)
    key = np.asarray(key, dtype=np.float32)
    value = np.asarray(value, dtype=np.float32)

    sg = 1.0 / np.sqrt(np.float32(HDG))
    consts = {
        "cqT": _arr_T(np.asarray(wq), 0.125),
        "ckT": _arr_T(np.asarray(wk)),
        "cvT": _arr_T(np.asarray(wv)),
        "cgqT": _arr_T(np.asarray(g_in_w)[:D], sg),
        "cgkT": _arr_T(np.asarray(g_in_w)[D:2 * D]),
        "cgvT": _arr_T(np.asarray(g_in_w)[2 * D:]),
        "cgoT": _arr_T(np.asarray(g_out_w), 0.3),
        "cwoT": _arr_T(np.asarray(wo)),
        "ident": np.eye(128, dtype=ml_dtypes.bfloat16),
    }
    # rel_ext [128, 1032]: rows 0-63 and 64-127 both hold
    # [d, u] = 8*rel_k[clip(u-256, 0, 512), d] (duplicated so lhsT/rhs base
    # partitions match for odd heads)
    idx = np.clip(np.arange(EXTW) - 256, 0, 2 * MAXREL)
    tbl = (8.0 * np.asarray(rel_k, dtype=np.float32)[idx]).T  # [64, 1025]
    relx = np.zeros((128, 1032), dtype=ml_dtypes.bfloat16)
    relx[:HD, :EXTW] = tbl.astype(ml_dtypes.bfloat16)
    relx[HD:2 * HD, :EXTW] = tbl.astype(ml_dtypes.bfloat16)
    consts["relx"] = relx

    bq, bk, bv = np.asarray(bq), np.asarray(bk), np.asarray(bv)
    bo, g_in_b, g_out_b = np.asarray(bo), np.asarray(g_in_b), np.asarray(g_out_b)
    flags = (bool(np.any(bq)), bool(np.any(bk)), bool(np.any(bv)),
             bool(np.any(g_in_b[:D])), bool(np.any(g_in_b[D:2 * D])),
             bool(np.any(g_in_b[2 * D:])), bool(np.any(g_out_b)), bool(np.any(bo)))
    if flags[0]:
        consts["bqT"] = _arr_bias(bq, 0.125)
    if flags[1]:
        consts["bkT"] = _arr_bias(bk)
    if flags[2]:
        consts["bvT"] = _arr_bias(bv, 0.7)   # deferred past softmax, 0.7 fold
    if flags[3]:
        consts["bgqT"] = _arr_bias(g_in_b[:D], sg)
    if flags[4]:
        consts["bgkT"] = _arr_bias(g_in_b[D:2 * D])
    if flags[5]:
        consts["bgvb"] = g_in_b[2 * D:].astype(np.float32)
    if flags[6]:
        consts["bgoT"] = _arr_bias(g_out_b, 0.3)
    if flags[7]:
        consts["bob"] = bo.astype(np.float32)

    if flags not in _cache:
        _cache[flags] = _build(flags)
    nc = _cache[flags]

    def _xT(x_core):
        # [BL, S, D] f32 -> [128, 4(ic), TOK] bf16 with [p, ic, t] = x[t, ic*128+p]
        xt = x_core.reshape(TOK, D).T.astype(ml_dtypes.bfloat16)   # [D, TOK]
        return np.ascontiguousarray(xt.reshape(4, 128, TOK).transpose(1, 0, 2))

    in_maps = []
    for c in range(NCORES):
        m = dict(consts)
        m["query"] = _xT(query[c * BL:(c + 1) * BL])
        m["key"] = _xT(key[c * BL:(c + 1) * BL])
        m["value"] = _xT(value[c * BL:(c + 1) * BL])
        in_maps.append(m)

    res = run_bass_kernel_spmd(nc, in_maps, core_ids=list(range(NCORES)))
    global LAST_RESULT
    LAST_RESULT = res
    out = np.concatenate(
        [r["out"].reshape(BL, S, D) for r in res.results], axis=0)
    return out.astype(np.float32)


LAST_RESULT = None


# revision 8
# speedup vs baseline: 1.1806x; 1.0837x over previous
"""Trainium2 Bass kernel for EnhancedTransformerNetwork (local rel-pos attention + global MHA).

Sharding: data-parallel over batch. 8 cores x 4 batches each; weights replicated.

Per-core algorithm (tokens = 4*512 = 2048, all matmuls bf16, PSUM f32):
  - inputs arrive host-transposed bf16 [din, tokens]
  - local:  qT8 = (wq/8)^T x^T (T-layout), kT (T), v natural.
            Per (batch, l-chunk): rel bias T2e for all 8 heads, with even/odd
            head matmuls row-tiled on array halves (K=64), evac'd into one
            wide tile, ONE DRAM write + ONE 3D diagonal-AP read (clip baked
            into the host rel_ext table) -> biasw[l, h, r].
            scores psum = qk (row-tiled pair, start) + ident-matmul(bias);
            exp on ACT with accum_out row sums; batched reciprocal per chunk;
            0.7/sums normalization on Pool; ONE DMA XBAR transpose per
            (batch, l-chunk) turns all 8 heads' exp [l, r] into expT [r, l];
            AV col-tiled per head pair (M=64 halves) -> localT [hd, l].
  - global: scoresT[r,l] = gkT^T gqT (K=128), exp; sums via ones-matmul;
            recip broadcast via fp32 K=1 PE outer product; AV with vg as lhsT
            -> gT [hdg, l] T-layout; g_out proj in T-layout (0.3 folded).
  - combT = localT + gprojT, final wo matmul, DMA out.
"""

import numpy as np
import ml_dtypes

import concourse.bass as bass
import concourse.mybir as mybir
import concourse.tile as tile
from concourse.bass_utils import run_bass_kernel_spmd

BF = mybir.dt.bfloat16
F32 = mybir.dt.float32
AF = mybir.ActivationFunctionType
ALU = mybir.AluOpType

D = 512
H = 8
HD = 64
HG = 4
HDG = 128
MAXREL = 256
S = 512
B = 32
NCORES = 8
BL = B // NCORES          # batches per core
TOK = BL * S              # tokens per core
EXTW = 1025               # rel_ext width: u' in [0, 1024], clip baked in
T2W = 640                 # per-chunk T2e width (639 used, padded to 640)
T2WIDE = H * T2W          # 5120: all 8 heads' T2e rows side by side

_cache = {}


def _split_wide_waits(nc, max_waits=1):
    """This walrus build supports only one sem wait per instruction; Tile's
    final drain carries one wait per logical proc. Hoist excess waits onto
    preceding same-engine NOPs (same-engine program order keeps semantics)."""
    f = nc.m.functions[0]
    created = {}
    for bb in f.blocks:
        for inst in bb.instructions:
            si = inst.sync_info
            if si is not None and len(si.on_wait) > max_waits:
                waits = list(si.on_wait)
                head, tail = waits[:-max_waits], waits[-max_waits:]
                nops = []
                for i in range(0, len(head), max_waits):
                    nop = nc.engines[inst.engine].nop(nofuse=True)
                    nop.ins.sync_info = mybir.SyncInfo(
                        on_wait=head[i:i + max_waits], on_update=[])
                    nops.append(nop.ins)
                si.on_wait = tail
                created[inst.name] = nops
    if not created:
        return
    names = {n.name for nops in created.values() for n in nops}
    for bb in f.blocks:
        out = []
        for inst in bb.instructions:
            if inst.name in names:
                continue
            if inst.name in created:
                out.extend(created[inst.name])
            out.append(inst)
        bb.instructions = out


def _build(flags, reps=1):
    """flags: (has_bq, has_bk, has_bv, has_bgq, has_bgk, has_bgv, has_bgo, has_bo)
    reps>1 replicates the whole computation on device (benchmarking only)."""
    has_bq, has_bk, has_bv, has_bgq, has_bgk, has_bgv, has_bgo, has_bo = flags
    nc = bass.Bass("TRN2", target_bir_lowering=False, debug=False,
                   num_devices=NCORES)

    # --- I/O ---
    x_q = nc.dram_tensor("query", [128, 4, TOK], BF, kind="ExternalInput")
    x_k = nc.dram_tensor("key", [128, 4, TOK], BF, kind="ExternalInput")
    x_v = nc.dram_tensor("value", [128, 4, TOK], BF, kind="ExternalInput")
    cw = {}
    for name in ["cqT", "ckT", "cvT", "cgqT", "cgkT", "cgvT", "cgoT", "cwoT"]:
        cw[name] = nc.dram_tensor(name, [128, 4, D], BF, kind="ExternalInput")
    relx_d = nc.dram_tensor("relx", [128, 1032], BF, kind="ExternalInput")
    ident_d = nc.dram_tensor("ident", [128, 128], BF, kind="ExternalInput")
    pbias_d = {}
    for name, has in [("bqT", has_bq), ("bkT", has_bk), ("bvT", has_bv),
                      ("bgqT", has_bgq), ("bgkT", has_bgk)]:
        if has:
            pbias_d[name] = nc.dram_tensor(name, [128, 4], F32, kind="ExternalInput")
    fbias_d = {}
    for name, has in [("bgvb", has_bgv), ("bob", has_bo)]:
        if has:
            fbias_d[name] = nc.dram_tensor(name, [D], F32, kind="ExternalInput")
    if has_bgo:
        pbias_d["bgoT"] = nc.dram_tensor("bgoT", [128, 4], F32, kind="ExternalInput")
    out_d = nc.dram_tensor("out", [TOK, D], F32, kind="ExternalOutput")

    # DRAM scratch for the T2e diagonal round trip, one per (b, lc), all heads
    t2scr = [[nc.dram_tensor(f"t2_{b}_{lc}", [128, T2WIDE], BF)
              for lc in range(4)] for b in range(BL)]

    with tile.TileContext(nc) as tc:
        import contextlib
        ctx = contextlib.ExitStack()
        with ctx:
            singles = ctx.enter_context(tc.tile_pool(name="singles", bufs=1))
            xload = ctx.enter_context(tc.tile_pool(name="xload", bufs=2))
            projs = ctx.enter_context(tc.tile_pool(name="projs", bufs=1))
            mid = ctx.enter_context(tc.tile_pool(name="mid", bufs=1))
            wide = ctx.enter_context(tc.tile_pool(name="wide", bufs=2))
            biaswp = ctx.enter_context(tc.tile_pool(name="biaswp", bufs=3))
            expp = ctx.enter_context(tc.tile_pool(name="expp", bufs=1))
            small = ctx.enter_context(tc.tile_pool(name="small", bufs=2))
            trans = ctx.enter_context(tc.tile_pool(name="trans", bufs=2))
            pwide = ctx.enter_context(tc.tile_pool(name="pwide", bufs=2, space="PSUM"))
            ps512 = ctx.enter_context(tc.tile_pool(name="ps512", bufs=4, space="PSUM"))

            # --- constants ---
            w_sb = {}
            for name in cw:
                t = singles.tile([128, 4, D], BF, tag=name)
                nc.sync.dma_start(t[:], cw[name][:, :, :])
                w_sb[name] = t
            relx = singles.tile([128, 1032], BF, tag="relx")
            nc.sync.dma_start(relx[:], relx_d[:, :])
            ident = singles.tile([128, 128], BF, tag="ident")
            nc.sync.dma_start(ident[:], ident_d[:, :])
            ones_bf = singles.tile([128, 1], BF, tag="ones_bf")
            nc.vector.memset(ones_bf[:], 1.0)
            ones_bfr = singles.tile([1, 128], BF, tag="ones_bfr")
            nc.vector.memset(ones_bfr[:], 1.0)
            pb_sb = {}
            for name, t in pbias_d.items():
                s = singles.tile([128, 4], F32, tag=name)
                nc.sync.dma_start(s[:], t[:, :])
                pb_sb[name] = s
            fb_sb = {}
            for name, t in fbias_d.items():
                s = singles.tile([128, D], F32, tag=name)
                nc.sync.dma_start(s[:], bass.AP(t, 0, [[0, 128], [1, D]]))
                fb_sb[name] = s

            evac_ctr = [0]

            def evac_copy(dst, src, bias_col=None, ratio=2):
                """PSUM->SBUF copy (+ optional per-partition bias), alternating
                DVE/ACT at ratio:1."""
                if bias_col is not None:
                    nc.vector.tensor_scalar_add(dst, src, bias_col)
                    return
                if evac_ctr[0] % (ratio + 1) < ratio:
                    nc.vector.tensor_copy(dst, src)
                else:
                    nc.scalar.activation(dst, src, AF.Copy)
                evac_ctr[0] += 1

            # ---- per-batch stage emitters (software pipeline across batches:
            # every engine's runtime stream follows emission order, so stages
            # are interleaved so no engine sits behind a long foreign dep) ----

            def load_x(b):
                tok0 = b * S
                xT = {}
                for key, xd in (("q", x_q), ("k", x_k), ("v", x_v)):
                    xT_t = xload.tile([128, 4, S], BF, tag=f"xT_{key}")
                    nc.sync.dma_start(xT_t[:], xd[:, :, tok0:tok0 + S])
                    xT[key] = xT_t
                return xT

            def projT(xT, dst, wname, bias_name):
                # dst [128(dout%128), 4(oc), 512(l)]  (T-layout out)
                for oc in range(4):
                    ps = ps512.tile([128, S], F32, tag="mm512")
                    for ic in range(4):
                        nc.tensor.matmul(ps[:], w_sb[wname][:, ic, oc * 128:(oc + 1) * 128],
                                         xT[:, ic, :], start=(ic == 0), stop=(ic == 3))
                    bias_col = pb_sb[bias_name][:, oc:oc + 1] if bias_name in pb_sb else None
                    evac_copy(dst[:, oc, :], ps[:], bias_col)

            def projN(xT, dst, wname):
                # dst [128(tok%128), 4(tc), 512(dout)]  (natural out)
                for tck in range(4):
                    ps = ps512.tile([128, D], F32, tag="mm512")
                    for ic in range(4):
                        nc.tensor.matmul(ps[:], xT[:, ic, tck * 128:(tck + 1) * 128],
                                         w_sb[wname][:, ic, :], start=(ic == 0), stop=(ic == 3))
                    evac_copy(dst[:, tck, :], ps[:])

            def t2e_block(st, lc):
                # T2e for all 8 heads of l-chunk lc + skew round trip
                b, qT8 = st["b"], st["qT8"]
                u0 = 385 - lc * 128
                t2w = wide.tile([128, H, T2W], BF, tag="t2w")
                for oc in range(4):
                    qsA = qT8[0:64, oc, lc * 128:(lc + 1) * 128]
                    qsB = qT8[64:128, oc, lc * 128:(lc + 1) * 128]
                    pwA = pwide.tile([128, 1024], F32, tag="pw")
                    pwB = pwide.tile([128, 1024], F32, tag="pw")
                    nc.tensor.matmul(pwA[:, 0:512], qsA, relx[0:64, u0:u0 + 512], start=True, stop=True)
                    nc.tensor.matmul(pwB[:, 0:512], qsB, relx[64:128, u0:u0 + 512], start=True, stop=True)
                    nc.tensor.matmul(pwA[:, 512:640], qsA, relx[0:64, u0 + 512:u0 + 640], start=True, stop=True)
                    nc.tensor.matmul(pwB[:, 512:640], qsB, relx[64:128, u0 + 512:u0 + 640], start=True, stop=True)
                    nc.vector.tensor_copy(t2w[:, 2 * oc, :], pwA[:, 0:T2W])
                    nc.vector.tensor_copy(t2w[:, 2 * oc + 1, :], pwB[:, 0:T2W])
                nc.sync.dma_start(t2scr[b][lc][:, :], t2w[:])
                # diagonal (skew) read: biasw[l, h, r] = T2e[l, h, r - l + 127]
                bw = biaswp.tile([128, H, S], BF, tag="biasw")
                nc.sync.dma_start(
                    bw[:],
                    bass.AP(t2scr[b][lc], 127, [[T2WIDE - 1, 128], [T2W, H], [1, S]]))
                st["biasw"][lc] = bw

            def scores_block(st, lc):
                qT8, kT = st["qT8"], st["kT"]
                sums, rec, expT = st["sums"], st["rec"], st["expT"]
                expn = wide.tile([128, H, S], BF, tag="expn")
                for oc in range(4):
                    qsA = qT8[0:64, oc, lc * 128:(lc + 1) * 128]
                    qsB = qT8[64:128, oc, lc * 128:(lc + 1) * 128]
                    psA = ps512.tile([128, S], F32, tag="mm512")
                    psB = ps512.tile([128, S], F32, tag="mm512")
                    nc.tensor.matmul(psA[:], qsA, kT[0:64, oc, :], start=True, stop=False)
                    nc.tensor.matmul(psB[:], qsB, kT[64:128, oc, :], start=True, stop=False)
                    nc.tensor.matmul(psA[:], ident[:], st["biasw"][lc][:, 2 * oc, :], start=False, stop=True)
                    nc.tensor.matmul(psB[:], ident[:], st["biasw"][lc][:, 2 * oc + 1, :], start=False, stop=True)
                    nc.scalar.activation(expn[:, 2 * oc, :], psA[:], AF.Exp,
                                         accum_out=sums[:, lc * 8 + 2 * oc:lc * 8 + 2 * oc + 1])
                    nc.scalar.activation(expn[:, 2 * oc + 1, :], psB[:], AF.Exp,
                                         accum_out=sums[:, lc * 8 + 2 * oc + 1:lc * 8 + 2 * oc + 2])
                # batched reciprocal + per-head 0.7/sums normalize on Pool
                nc.vector.reciprocal(rec[:, lc * 8:(lc + 1) * 8], sums[:, lc * 8:(lc + 1) * 8])
                for h in range(H):
                    nc.gpsimd.tensor_scalar(expn[:, h, :], expn[:, h, :],
                                            rec[:, lc * 8 + h:lc * 8 + h + 1], 0.7,
                                            ALU.mult, ALU.mult)
                # one XBAR transpose: expT[r%128, h, rc, l-chunk] <- expn[l, h, r]
                nc.sync.dma_start(expT[:, :, :, lc * 128:(lc + 1) * 128], expn[:, :, :],
                                  transpose=True)

            def gsc(st, hg):
                # global scoresT + exp
                expTg = trans.tile([128, 4, S], BF, tag="expTg")
                for rc in range(4):
                    ps = ps512.tile([128, S], F32, tag="mm512")
                    nc.tensor.matmul(ps[:], st["gkT"][:, hg, rc * 128:(rc + 1) * 128],
                                     st["gqT"][:, hg, :], start=True, stop=True)
                    nc.scalar.activation(expTg[:, rc, :], ps[:], AF.Exp)
                st["expTg"][hg] = expTg

            def gones(st, hg):
                # partition sums via ones-matmul, recip, broadcast
                expTg = st["expTg"][hg]
                psum = ps512.tile([128, S], F32, tag="mm512")
                for rc in range(4):
                    nc.tensor.matmul(psum[0:1, :], ones_bf[:], expTg[:, rc, :],
                                     start=(rc == 0), stop=(rc == 3))
                recip_g = trans.tile([1, S], BF, tag="recip_g")
                with nc.allow_low_precision(reason="bf16 recip feeds bf16 bcast matmul; error cancels with bf16 exp sums"):
                    nc.vector.reciprocal(recip_g[:], psum[0:1, :])
                pbc = ps512.tile([128, S], F32, tag="mm512")
                nc.tensor.matmul(pbc[:], ones_bfr[:], recip_g[:], start=True, stop=True)
                recipB = trans.tile([128, S], F32, tag="recipB")
                nc.scalar.activation(recipB[:], pbc[:], AF.Copy)
                st["recipB"][hg] = recipB

            def gav(st, hg):
                expTg, vg_n, gT = st["expTg"][hg], st["vg_n"], st["gT"]
                pgT = ps512.tile([128, S], F32, tag="mm512")
                for rc in range(4):
                    nc.tensor.matmul(pgT[:], vg_n[:, rc, hg * 128:(hg + 1) * 128],
                                     expTg[:, rc, :], start=(rc == 0), stop=(rc == 3))
                nc.vector.tensor_tensor(gT[:, hg, :], pgT[:], st["recipB"][hg], ALU.mult)
                if "bgvb" in fb_sb:
                    nc.vector.tensor_scalar_add(gT[:, hg, :], gT[:, hg, :],
                                                fb_sb["bgvb"][hg * 128:(hg + 1) * 128, 0:1])

            def av_block(st, oc):
                v_n, expT, localT = st["v_n"], st["expT"], st["localT"]
                pav = ps512.tile([128, S], F32, tag="mm512")
                for rc in range(4):
                    nc.tensor.matmul(pav[0:64, :], v_n[:, rc, oc * 128:oc * 128 + 64],
                                     expT[:, 2 * oc, rc, :], start=(rc == 0), stop=(rc == 3))
                    nc.tensor.matmul(pav[64:128, :], v_n[:, rc, oc * 128 + 64:(oc + 1) * 128],
                                     expT[:, 2 * oc + 1, rc, :], start=(rc == 0), stop=(rc == 3))
                if "bvT" in pb_sb:
                    # deferred v bias (scaled 0.7 on host): out = AV*0.7/sums + 0.7*bv
                    nc.vector.tensor_scalar_add(localT[:, oc, :], pav[:],
                                                pb_sb["bvT"][:, oc:oc + 1])
                else:
                    nc.vector.tensor_copy(localT[:, oc, :], pav[:])

            def combine_wo(st):
                b, gT, localT = st["b"], st["gT"], st["localT"]
                tok0 = b * S
                combT = mid.tile([128, 4, S], BF, tag="combT")
                for oc in range(4):
                    ps = ps512.tile([128, S], F32, tag="mm512")
                    for ic in range(4):
                        nc.tensor.matmul(ps[:], w_sb["cgoT"][:, ic, oc * 128:(oc + 1) * 128],
                                         gT[:, ic, :], start=(ic == 0), stop=(ic == 3))
                    if "bgoT" in pb_sb:
                        nc.vector.tensor_scalar_add(ps[:], ps[:], pb_sb["bgoT"][:, oc:oc + 1])
                    nc.vector.tensor_tensor(combT[:, oc, :], ps[:], localT[:, oc, :], ALU.add)
                for tck in range(4):
                    ps = ps512.tile([128, D], F32, tag="mm512")
                    for ic in range(4):
                        nc.tensor.matmul(ps[:], combT[:, ic, tck * 128:(tck + 1) * 128],
                                         w_sb["cwoT"][:, ic, :], start=(ic == 0), stop=(ic == 3))
                    osb = trans.tile([128, D], F32, tag="osb")
                    if "bob" in fb_sb:
                        nc.vector.tensor_tensor(osb[:], ps[:], fb_sb["bob"][:], ALU.add)
                    else:
                        nc.scalar.activation(osb[:], ps[:], AF.Copy)
                    nc.sync.dma_start(out_d[tok0 + tck * 128: tok0 + (tck + 1) * 128, :], osb[:])

            def alloc_state(b):
                return {
                    "b": b,
                    "qT8": projs.tile([128, 4, S], BF, tag="qT8", name="qT8"),
                    "kT": projs.tile([128, 4, S], BF, tag="kT", name="kT"),
                    "gqT": projs.tile([128, 4, S], BF, tag="gqT", name="gqT"),
                    "gkT": projs.tile([128, 4, S], BF, tag="gkT", name="gkT"),
                    "v_n": projs.tile([128, 4, D], BF, tag="v_n", name="v_n"),
                    "vg_n": projs.tile([128, 4, D], BF, tag="vg_n", name="vg_n"),
                    "localT": mid.tile([128, 4, S], BF, tag="localT", name="localT"),
                    "gT": mid.tile([128, 4, S], BF, tag="gT", name="gT"),
                    "expT": expp.tile([128, H, 4, S], BF, tag="expT", name="expT"),
                    "sums": small.tile([128, 32], F32, tag="sums", name="sums"),
                    "rec": small.tile([128, 32], F32, tag="rec", name="rec"),
                    "biasw": [None] * 4,
                    "expTg": [None] * HG,
                    "recipB": [None] * HG,
                }

            for rep in range(reps):
                # prologue: batch 0 projections
                xT0 = load_x(0)
                st = alloc_state(0)
                projT(xT0["q"], st["qT8"], "cqT", "bqT")
                projT(xT0["k"], st["kT"], "ckT", "bkT")
                projT(xT0["q"], st["gqT"], "cgqT", "bgqT")
                projT(xT0["k"], st["gkT"], "cgkT", "bgkT")
                projN(xT0["v"], st["v_n"], "cvT")
                projN(xT0["v"], st["vg_n"], "cgvT")

                for b in range(BL):
                    # prefetch next batch's inputs early
                    xT_n = load_x(b + 1) if b + 1 < BL else None
                    # T2e round trips with global stages staggered between
                    # them (global PE/ACT work fills the skew-DMA latency)
                    t2e_block(st, 0)
                    gsc(st, 0)
                    t2e_block(st, 1)
                    gones(st, 0)
                    gsc(st, 1)
                    t2e_block(st, 2)
                    gav(st, 0)
                    gones(st, 1)
                    gsc(st, 2)
                    t2e_block(st, 3)
                    gav(st, 1)
                    gones(st, 2)
                    gsc(st, 3)
                    scores_block(st, 0)
                    gav(st, 2)
                    gones(st, 3)
                    scores_block(st, 1)
                    gav(st, 3)
                    scores_block(st, 2)
                    scores_block(st, 3)
                    # next batch's T-projections: PE fill while ACT/POOL/DMA
                    # finish the exp/normalize/transpose chain above
                    if xT_n is not None:
                        st_n = alloc_state(b + 1)
                        projT(xT_n["q"], st_n["qT8"], "cqT", "bqT")
                        projT(xT_n["k"], st_n["kT"], "ckT", "bkT")
                        projT(xT_n["q"], st_n["gqT"], "cgqT", "bgqT")
                        projT(xT_n["k"], st_n["gkT"], "cgkT", "bgkT")
                    for oc in range(4):
                        av_block(st, oc)
                    # v projections after AV (their dst buffers are WAR-tied
                    # to this batch's AV reads)
                    if xT_n is not None:
                        projN(xT_n["v"], st_n["v_n"], "cvT")
                        projN(xT_n["v"], st_n["vg_n"], "cgvT")
                    combine_wo(st)
                    if xT_n is not None:
                        st = st_n

    _split_wide_waits(nc)
    return nc


def _arr_T(w, scale=1.0):
    """torch Linear weight [dout, din] -> lhsT const [128, 4, dout] bf16 with
    [p, ic, o] = w[o, ic*128+p]*scale."""
    wt = (w.T.astype(np.float64) * scale).astype(np.float32)
    return np.ascontiguousarray(
        wt.reshape(4, 128, wt.shape[1]).transpose(1, 0, 2)).astype(ml_dtypes.bfloat16)


def _arr_bias(bvec, scale=1.0):
    """bias [512] -> [128, 4] f32 with [p, oc] = b[oc*128+p]*scale."""
    return np.ascontiguousarray(
        (np.asarray(bvec) * scale).astype(np.float32).reshape(4, 128).T)


def kernel(query, key, value, wq, bq, wk, bk, wv, bv, wo, bo, rel_k,
           g_in_w, g_in_b, g_out_w, g_out_b):
    query = np.asarray(query, dtype=np.float32)
    key = np.asarray(key, dtype=np.float32)
    value = np.asarray(value, dtype=np.float32)

    sg = 1.0 / np.sqrt(np.float32(HDG))
    consts = {
        "cqT": _arr_T(np.asarray(wq), 0.125),
        "ckT": _arr_T(np.asarray(wk)),
        "cvT": _arr_T(np.asarray(wv)),
        "cgqT": _arr_T(np.asarray(g_in_w)[:D], sg),
        "cgkT": _arr_T(np.asarray(g_in_w)[D:2 * D]),
        "cgvT": _arr_T(np.asarray(g_in_w)[2 * D:]),
        "cgoT": _arr_T(np.asarray(g_out_w), 0.3),
        "cwoT": _arr_T(np.asarray(wo)),
        "ident": np.eye(128, dtype=ml_dtypes.bfloat16),
    }
    # rel_ext [128, 1032]: rows 0-63 and 64-127 both hold
    # [d, u] = 8*rel_k[clip(u-256, 0, 512), d] (duplicated so lhsT/rhs base
    # partitions match for odd heads)
    idx = np.clip(np.arange(EXTW) - 256, 0, 2 * MAXREL)
    tbl = (8.0 * np.asarray(rel_k, dtype=np.float32)[idx]).T  # [64, 1025]
    relx = np.zeros((128, 1032), dtype=ml_dtypes.bfloat16)
    relx[:HD, :EXTW] = tbl.astype(ml_dtypes.bfloat16)
    relx[HD:2 * HD, :EXTW] = tbl.astype(ml_dtypes.bfloat16)
    consts["relx"] = relx

    bq, bk, bv = np.asarray(bq), np.asarray(bk), np.asarray(bv)
    bo, g_in_b, g_out_b = np.asarray(bo), np.asarray(g_in_b), np.asarray(g_out_b)
    flags = (bool(np.any(bq)), bool(np.any(bk)), bool(np.any(bv)),
             bool(np.any(g_in_b[:D])), bool(np.any(g_in_b[D:2 * D])),
             bool(np.any(g_in_b[2 * D:])), bool(np.any(g_out_b)), bool(np.any(bo)))
    if flags[0]:
        consts["bqT"] = _arr_bias(bq, 0.125)
    if flags[1]:
        consts["bkT"] = _arr_bias(bk)
    if flags[2]:
        consts["bvT"] = _arr_bias(bv, 0.7)   # deferred past softmax, 0.7 fold
    if flags[3]:
        consts["bgqT"] = _arr_bias(g_in_b[:D], sg)
    if flags[4]:
        consts["bgkT"] = _arr_bias(g_in_b[D:2 * D])
    if flags[5]:
        consts["bgvb"] = g_in_b[2 * D:].astype(np.float32)
    if flags[6]:
        consts["bgoT"] = _arr_bias(g_out_b, 0.3)
    if flags[7]:
        consts["bob"] = bo.astype(np.float32)

    if flags not in _cache:
        _cache[flags] = _build(flags)
    nc = _cache[flags]

    def _xT(x_core):
        # [BL, S, D] f32 -> [128, 4(ic), TOK] bf16 with [p, ic, t] = x[t, ic*128+p]
        xt = x_core.reshape(TOK, D).T.astype(ml_dtypes.bfloat16)   # [D, TOK]
        return np.ascontiguousarray(xt.reshape(4, 128, TOK).transpose(1, 0, 2))

    in_maps = []
    for c in range(NCORES):
        m = dict(consts)
        m["query"] = _xT(query[c * BL:(c + 1) * BL])
        m["key"] = _xT(key[c * BL:(c + 1) * BL])
        m["value"] = _xT(value[c * BL:(c + 1) * BL])
        in_maps.append(m)

    res = run_bass_kernel_spmd(nc, in_maps, core_ids=list(range(NCORES)))
    global LAST_RESULT
    LAST_RESULT = res
    out = np.concatenate(
        [r["out"].reshape(BL, S, D) for r in res.results], axis=0)
    return out.astype(np.float32)


LAST_RESULT = None
